# revision 10
# baseline (speedup 1.0000x reference)
"""DCNv2 deformable ROI pooling on 8 Trainium2 NeuronCores (v2).

Per-bin the 4x4 bilinear sample grid is separable (y outer-product x), so
each ROI's pooled output is one small accumulated matmul
    out[49 bins, 256 ch] = M[49, K] @ PatchFlat[K, 256]
where K indexes a feature-map patch window covering the ROI's samples and
M = alpha (x) beta comes from host-precomputed per-axis interpolation
weights.  ROIs (dim 0) are sharded across the 8 cores (512 = 8 x 64, one
ROI per slot per core); the channels-last bf16 feature map is replicated.

v2 vs v1:
 - exact per-slot patch sizes (size-sorted deal; slot R/L = max over its
   8 ROIs) instead of coarse size classes
 - chunk geometry (G col-groups, nk chunks) chosen per slot by a DMA/PE
   cost model fitted to microbenchmarks; single-packet DMAs
 - slots processed in pairs on disjoint PE quadrants: slot A on array
   rows/cols 0-63 (psum partitions 0-48), slot B on rows/cols 64-127
   (psum partitions 64-112) -> 2x matmul concurrency, and pair MT blocks
   stack vertically in one free range (halves MT traffic)
 - bf16 packed output (49x256 per slot), cast to f32 on host
"""

import numpy as np

import concourse.bass as bass
import concourse.mybir as mybir
import concourse.tile as tile
from concourse import bacc
import concourse.bass_utils as bass_utils

B, C, H, W = 4, 256, 128, 128
N_ROIS = 512
P = 7
PP = P * P
SCALE = np.float32(0.0625)
RATIO = 4
GAMMA = np.float32(0.1)
N_CORES = 8
N_SLOTS = N_ROIS // N_CORES      # 64
N_PAIRS = N_SLOTS // 2           # 32
GROUP_PAIRS = 8                  # pairs per mt chunk / output flush group
N_GROUPS = N_PAIRS // GROUP_PAIRS

PATCH_BUFS = 14
PSUM_BUFS = 8
SINGLE_PACKET = False

_f32 = np.float32


def _t_desc(nbytes):
    """ns per descriptor per SDMA engine (HW-measured)."""
    return 12.0 + 0.0826 * nbytes


def _geom(R, L):
    """Chunk geometry for an R x L patch: partition p = r*G + s holds
    pixels (row r, col s*nk + k) for chunk k; Q = G*R <= 64 partitions,
    L padded to G*nk.  Picks nk minimizing modeled DMA+PE cost:
    ring-enqueue (~13ns/desc/ring) + SDMA service + PE rounds (213ns
    cold back-to-back, ~halved by pair overlap)."""
    Gcap = max(1, 128 // R)
    nk_lo = -(-L // Gcap)
    best = None
    for nk in range(nk_lo, L + 1):
        G = -(-L // nk)
        nke = -(-L // G)
        Q = G * R
        cost = (Q * 8.0
                + Q * _t_desc(nke * 2 * C) / 16.0
                + nke * 213.0 * 0.7)
        if best is None or cost < best[0] - 1e-9:
            best = (cost, G, nke)
        if nke <= 1:
            break
    _, G, nk = best
    return G, nk, G * R, G * nk


def _prep(rois, offset):
    """Dense per-axis interpolation weights + per-ROI sample bounds."""
    n = rois.shape[0]
    bidx = rois[:, 0].astype(np.int32)
    x1 = rois[:, 1] * SCALE - _f32(0.5)
    y1 = rois[:, 2] * SCALE - _f32(0.5)
    x2 = rois[:, 3] * SCALE - _f32(0.5)
    y2 = rois[:, 4] * SCALE - _f32(0.5)
    rw = np.maximum(x2 - x1, _f32(1.0))
    rh = np.maximum(y2 - y1, _f32(1.0))
    bw = rw / _f32(P)
    bh = rh / _f32(P)
    off = offset.reshape(n, 2, P, P).astype(np.float32)
    off_x = GAMMA * rw[:, None, None] * off[:, 0]
    off_y = GAMMA * rh[:, None, None] * off[:, 1]
    ph = np.arange(P, dtype=np.float32)
    s = ((np.arange(RATIO, dtype=np.float32) + _f32(0.5)) / _f32(RATIO))
    ybase = y1[:, None, None] + ph[None, :, None] * bh[:, None, None] + off_y
    xbase = x1[:, None, None] + ph[None, None, :] * bw[:, None, None] + off_x
    ys = ybase[..., None] + s[None, None, None, :] * bh[:, None, None, None]
    xs = xbase[..., None] + s[None, None, None, :] * bw[:, None, None, None]
    vy = (ys > -1.0) & (ys < H)
    vx = (xs > -1.0) & (xs < W)
    yc = np.clip(ys, _f32(0.0), _f32(H - 1))
    xc = np.clip(xs, _f32(0.0), _f32(W - 1))
    y0 = np.floor(yc).astype(np.int32)
    x0 = np.floor(xc).astype(np.int32)
    y1i = np.minimum(y0 + 1, H - 1)
    x1i = np.minimum(x0 + 1, W - 1)
    ly = (yc - y0).astype(np.float32)
    lx = (xc - x0).astype(np.float32)
    hy = _f32(1.0) - ly
    hx = _f32(1.0) - lx

    npp = n * PP
    alpha_d = np.zeros((npp, H), np.float32)
    beta_d = np.zeros((npp, W), np.float32)
    rows = np.repeat(np.arange(npp), RATIO)
    inv = _f32(1.0 / RATIO)
    np.add.at(alpha_d, (rows, y0.reshape(npp, RATIO).ravel()),
              (np.where(vy, hy, 0).reshape(npp, RATIO) * inv).ravel())
    np.add.at(alpha_d, (rows, y1i.reshape(npp, RATIO).ravel()),
              (np.where(vy, ly, 0).reshape(npp, RATIO) * inv).ravel())
    np.add.at(beta_d, (rows, x0.reshape(npp, RATIO).ravel()),
              (np.where(vx, hx, 0).reshape(npp, RATIO) * inv).ravel())
    np.add.at(beta_d, (rows, x1i.reshape(npp, RATIO).ravel()),
              (np.where(vx, lx, 0).reshape(npp, RATIO) * inv).ravel())

    ymin = np.minimum(y0.reshape(n, -1).min(axis=1), H - 1)
    ymax = np.minimum(y1i.reshape(n, -1).max(axis=1), H - 1)
    xmin = np.minimum(x0.reshape(n, -1).min(axis=1), W - 1)
    xmax = np.minimum(x1i.reshape(n, -1).max(axis=1), W - 1)
    return (bidx, ymin, ymax, xmin, xmax,
            alpha_d.reshape(n, PP, H), beta_d.reshape(n, PP, W))


def _mt_block(alpha_w, beta_w, G, nk):
    """[PP, R] x [PP, Lp] weights -> device MT block [Q, nk*PP]."""
    R = alpha_w.shape[1]
    Q = G * R
    p = np.arange(Q)
    a = alpha_w[:, p // G]                                    # [PP, Q]
    l_idx = (p[:, None] % G) * nk + np.arange(nk)[None, :]    # [Q, nk]
    b = beta_w[:, l_idx]                                      # [PP, Q, nk]
    mtb = a.T[:, None, :] * b.transpose(1, 2, 0)              # [Q, nk, PP]
    return mtb.reshape(Q, nk * PP).astype(np.float32)


def _layout(sr, sl):
    """Deal size-sorted ROIs to (slot, core); slot geometry from maxima.
    Returns slot_roi [N_SLOTS, N_CORES] and per-slot specs."""
    key = sl.astype(np.int64) * 1000 + sr
    order = np.argsort(-key, kind="stable")
    slot_roi = order.reshape(N_SLOTS, N_CORES)
    specs = []
    for s_ in range(N_SLOTS):
        grp = slot_roi[s_]
        R = int(sr[grp].max())
        L = int(sl[grp].max())
        G, nk, Q, Lp = _geom(R, L)
        specs.append((R, L, G, nk, Q, Lp))
    so = sorted(range(N_SLOTS), key=lambda s_: (-specs[s_][3], -specs[s_][4]))
    return slot_roi[so], [specs[s_] for s_ in so]


def _pair_meta(specs):
    """Per-slot MT free offsets (blocks side by side, all at partition
    base 0) and per-group chunk bounds."""
    ws = [PP * specs[s_][3] for s_ in range(N_SLOTS)]
    fo = np.concatenate([[0], np.cumsum(ws)]).astype(int)
    gb = [int(fo[2 * GROUP_PAIRS * g]) for g in range(N_GROUPS)] \
        + [int(fo[-1])]
    return fo, gb


def _max_off(spec):
    R, L, G, nk, Q, Lp = spec
    return (((B - 1) * H + (H - R)) * W + (W - Lp)) * C


_NC_CACHE = {}


def _build_kernel(specs):
    key = (tuple(specs), SINGLE_PACKET)
    if key in _NC_CACHE:
        return _NC_CACHE[key]
    fo, gb = _pair_meta(specs)
    mt_free = gb[-1]
    gw_max = max(gb[g + 1] - gb[g] for g in range(N_GROUPS))
    bf16 = mybir.dt.bfloat16

    nc = bacc.Bacc("TRN2", target_bir_lowering=False, debug=False,
                   num_devices=N_CORES)
    xt = nc.dram_tensor("xt", [B * H * W, C], bf16,
                        kind="ExternalInput").ap()
    mt = nc.dram_tensor("mt", [128, mt_free], bf16,
                        kind="ExternalInput").ap()
    po = nc.dram_tensor("po", [1, N_SLOTS], mybir.dt.int32,
                        kind="ExternalInput").ap()
    # out[g]: rows 0:49 = slot A (pair pj), rows 49:98 = slot B
    out = nc.dram_tensor("out", [N_GROUPS, 2 * PP, GROUP_PAIRS * C],
                         bf16, kind="ExternalOutput").ap()

    OFFC = 8                       # offsets per register-load batch
    n_chunks = N_PAIRS // OFFC     # per engine
    with tile.TileContext(nc) as tc:
        with (
            tc.tile_pool(name="offp", bufs=1) as offp,
            tc.tile_pool(name="mtp", bufs=3) as mtp,
            tc.tile_pool(name="patchp", bufs=PATCH_BUFS) as patchp,
            tc.tile_pool(name="outp", bufs=2) as outp,
            tc.tile_pool(name="psump", bufs=PSUM_BUFS, space="PSUM") as psump,
        ):
            offs = offp.tile([1, N_SLOTS], mybir.dt.int32)
            nc.sync.dma_start(offs[:, :], po[:, :])

            mt_tiles = {}

            def load_mt(g):
                w = gb[g + 1] - gb[g]
                t = mtp.tile([128, gw_max], bf16, tag="mt")
                eng = nc.scalar if g % 2 == 0 else nc.sync
                eng.dma_start(t[:, 0:w], mt[:, gb[g]:gb[g + 1]])
                mt_tiles[g] = t

            off_vals = {}

            def load_offs(ring, ci):
                """Load OFFC offsets into ring-engine registers.  po
                layout: positions [OFFC*c, ...) = A slots of pairs
                OFFC*c.. (SP ring); N_PAIRS + same = B slots (ACT)."""
                base = OFFC * ci + (0 if ring == 0 else N_PAIRS)
                slots = [2 * (OFFC * ci + j) + ring for j in range(OFFC)]
                eng = (mybir.EngineType.SP if ring == 0
                       else mybir.EngineType.Activation)
                _, vs = nc.values_load_multi_w_load_instructions(
                    offs[0:1, base:base + OFFC],
                    engines=[eng],
                    min_val=0,
                    max_val=max(_max_off(specs[s_]) for s_ in slots),
                    skip_runtime_bounds_check=True)
                off_vals.update(zip(slots, vs))

            load_mt(0)   # scalar ring, overlaps the po/offset chain on SP
            load_offs(0, 0)
            load_offs(1, 0)
            load_mt(1)

            for g in range(N_GROUPS):
                osb = outp.tile([128, GROUP_PAIRS * C], bf16, tag="osb")
                mt_sb = mt_tiles.pop(g)
                for pj in range(GROUP_PAIRS):
                    pr = g * GROUP_PAIRS + pj
                    if pr % OFFC == 2 and pr // OFFC + 1 < n_chunks:
                        load_offs(0, pr // OFFC + 1)
                    if pr % OFFC == 5 and pr // OFFC + 1 < n_chunks:
                        load_offs(1, pr // OFFC + 1)
                    sA, sB = 2 * pr, 2 * pr + 1
                    RA, LA, GA, nkA, QA, LpA = specs[sA]
                    RB, LB, GB, nkB, QB, LpB = specs[sB]
                    nkm = max(nkA, nkB)
                    fA = fo[sA] - gb[g]
                    fB = fo[sB] - gb[g]
                    ptA = patchp.tile([QA, nkA * C], bf16, tag="patch")
                    ptB = patchp.tile([QB, nkB * C], bf16, tag="patch")
                    srcA = bass.AP(xt.tensor, off_vals[sA],
                                   [[W * C, RA], [1, LpA * C]])
                    nc.sync.dma_start(ptA[:, :], srcA,
                                      single_packet=SINGLE_PACKET)
                    srcB = bass.AP(xt.tensor, off_vals[sB],
                                   [[W * C, RB], [1, LpB * C]])
                    nc.scalar.dma_start(ptB[:, :], srcB,
                                        single_packet=SINGLE_PACKET)
                    ps = psump.tile([128, C], mybir.dt.float32, space="PSUM")
                    # interleave A/B chunks: matmuls start in program
                    # order, so A0,B0,A1,B1,... lets the two column-tile
                    # halves run concurrently (pair span ~= max(nk) rounds)
                    for k in range(nkm):
                        if k < nkA:
                            nc.tensor.matmul(
                                ps[0:PP, :],
                                lhsT=mt_sb[0:QA,
                                           fA + k * PP:fA + (k + 1) * PP],
                                rhs=ptA[:, k * C:(k + 1) * C],
                                start=(k == 0), stop=(k == nkA - 1),
                                tile_position=(0, 0))
                        if k < nkB:
                            nc.tensor.matmul(
                                ps[64:64 + PP, :],
                                lhsT=mt_sb[0:QB,
                                           fB + k * PP:fB + (k + 1) * PP],
                                rhs=ptB[:, k * C:(k + 1) * C],
                                start=(k == 0), stop=(k == nkB - 1),
                                tile_position=(0, 64))
                    cs = pj * C
                    nc.vector.tensor_copy(osb[0:PP, cs:cs + C], ps[0:PP, :])
                    nc.vector.tensor_copy(osb[64:64 + PP, cs:cs + C],
                                          ps[64:64 + PP, :])
                if g + 2 < N_GROUPS:
                    load_mt(g + 2)
                e1 = nc.sync if g % 2 == 0 else nc.scalar
                e2 = nc.scalar if g % 2 == 0 else nc.sync
                e1.dma_start(out[g][0:PP, :], osb[0:PP, :])
                e2.dma_start(out[g][PP:2 * PP, :], osb[64:64 + PP, :])
    nc.compile()
    _NC_CACHE[key] = nc
    return nc


def _reference_fallback(x, rois, offset, idx):
    """Exact numpy replica of the reference (used by test.py; safety net)."""
    n = len(idx)
    if n == 0:
        return np.zeros((0, C, P, P), np.float32)
    rois = rois[idx]
    offset = offset[idx]
    bidx = rois[:, 0].astype(np.int32)
    x1 = rois[:, 1] * SCALE - _f32(0.5)
    y1 = rois[:, 2] * SCALE - _f32(0.5)
    x2 = rois[:, 3] * SCALE - _f32(0.5)
    y2 = rois[:, 4] * SCALE - _f32(0.5)
    rw = np.maximum(x2 - x1, _f32(1.0))
    rh = np.maximum(y2 - y1, _f32(1.0))
    bw, bh = rw / _f32(P), rh / _f32(P)
    off = offset.reshape(n, 2, P, P)
    off_x = GAMMA * rw[:, None, None] * off[:, 0]
    off_y = GAMMA * rh[:, None, None] * off[:, 1]
    ph = np.arange(P, dtype=np.float32)
    s = (np.arange(RATIO, dtype=np.float32) + _f32(0.5)) / _f32(RATIO)
    ybase = y1[:, None, None] + ph[None, :, None] * bh[:, None, None] + off_y
    xbase = x1[:, None, None] + ph[None, None, :] * bw[:, None, None] + off_x
    ys = ybase[..., None, None] + s[:, None][None, None, None] * bh[:, None, None, None, None]
    xs = xbase[..., None, None] + s[None, :][None, None, None] * bw[:, None, None, None, None]
    ys, xs = np.broadcast_arrays(ys, xs)
    valid = (ys > -1.0) & (ys < H) & (xs > -1.0) & (xs < W)
    yc = np.clip(ys, 0.0, _f32(H - 1))
    xc = np.clip(xs, 0.0, _f32(W - 1))
    y0 = np.floor(yc).astype(np.int32)
    x0 = np.floor(xc).astype(np.int32)
    y1i = np.minimum(y0 + 1, H - 1)
    x1i = np.minimum(x0 + 1, W - 1)
    ly = (yc - y0).astype(np.float32)
    lx = (xc - x0).astype(np.float32)
    hy, hx = _f32(1.0) - ly, _f32(1.0) - lx
    b = bidx[:, None, None, None, None]
    val = ((hy * hx)[..., None] * x[b, :, y0, x0]
           + (hy * lx)[..., None] * x[b, :, y0, x1i]
           + (ly * hx)[..., None] * x[b, :, y1i, x0]
           + (ly * lx)[..., None] * x[b, :, y1i, x1i])
    val = np.where(valid[..., None], val, _f32(0.0))
    return val.mean(axis=(3, 4)).transpose(0, 3, 1, 2)


def kernel(input, rois, offset):
    import ml_dtypes
    input = np.asarray(input, dtype=np.float32)
    rois = np.asarray(rois, dtype=np.float32)
    offset = np.asarray(offset, dtype=np.float32)

    xt = np.ascontiguousarray(
        input.transpose(0, 2, 3, 1).reshape(B * H * W, C)
    ).astype(ml_dtypes.bfloat16)
    bidx, ymin, ymax, xmin, xmax, alpha_d, beta_d = _prep(rois, offset)
    sr = (ymax - ymin + 1).astype(np.int64)
    sl = (xmax - xmin + 1).astype(np.int64)
    slot_roi, specs = _layout(sr, sl)
    fo, gb = _pair_meta(specs)
    mt_free = gb[-1]

    mt_all = np.zeros((N_CORES, 128, mt_free), ml_dtypes.bfloat16)
    po_all = np.zeros((N_CORES, N_SLOTS), np.int32)
    for slot in range(N_SLOTS):
        R, L, G, nk, Q, Lp = specs[slot]
        pr, ab = divmod(slot, 2)
        pos = pr if ab == 0 else N_PAIRS + pr
        for core in range(N_CORES):
            ridx = int(slot_roi[slot, core])
            py0 = min(max(int(ymin[ridx]), 0), H - R)
            px0 = min(max(int(xmin[ridx]), 0), W - Lp)
            blk = _mt_block(alpha_d[ridx, :, py0:py0 + R],
                            beta_d[ridx, :, px0:px0 + Lp], G, nk)
            mt_all[core, 0:Q, fo[slot]:fo[slot] + nk * PP] = blk
            po_all[core, pos] = ((int(bidx[ridx]) * H + py0) * W + px0) * C

    nc = _build_kernel(tuple(specs))
    in_maps = [{"xt": xt, "mt": mt_all[c], "po": po_all[c][None, :]}
               for c in range(N_CORES)]
    kernel.last_nc = nc
    kernel.last_in_maps = in_maps
    runner = getattr(kernel, "runner", None)
    if runner is not None:
        res = runner(nc, in_maps)
    else:
        res = bass_utils.run_bass_kernel_spmd(nc, in_maps,
                                              core_ids=list(range(N_CORES)))
    kernel.last_results = res

    out = np.zeros((N_ROIS, C, P, P), np.float32)
    for slot in range(N_SLOTS):
        pr, ab = divmod(slot, 2)
        g, pj = divmod(pr, GROUP_PAIRS)
        r0 = 0 if ab == 0 else PP
        for core in range(N_CORES):
            ridx = int(slot_roi[slot, core])
            dev = res.results[core]["out"]
            blk = np.asarray(dev[g][r0:r0 + PP, pj * C:(pj + 1) * C],
                             dtype=np.float32)
            out[ridx] = blk.T.reshape(C, P, P)
    return np.ascontiguousarray(out)
